# revision 1
# baseline (speedup 1.0000x reference)
"""Trainium2 Bass kernel for nn_CZT_prop: chirp-z (Bluestein) optical propagation.

Math: per wavelength the two Bluestein CZTs are dense 1024x1024 complex matmuls
with the SAME chirp matrix T = diag(g) E diag(b), E[k,j] = exp(i*alpha*k*j).
    out = F0 . (T (field.F) T^T) * Z*ODX*ODY*wl        (. = elementwise)
Device computes X1 = U0^T Eh (cols = this core's half), X2 = Ehat^T X1,
y = F0 . X2, with all row/col orders sigma-permuted (sigma = [0..511,
1023..512]) so the 4-fold symmetric RS kernels F/F0 are consumed as plain
quadrant tiles.

This version precomputes every transcendental table on the HOST (f64 trig,
single fp16 rounding): U0 = field.F (with all pow2 scale factors folded in),
the chirp matrix E, and the output kernel F0. The device runs ONLY:
  - 384 fp16 matmuls (3-multiplication Karatsuba complex matmul, N=512)
  - DVE psum combines (X1r=P1-P2, X1i=P3-P1-P2, X1s=X1r+X1i) - plain casts,
    no scaling ops anywhere (scales folded into host tables; all pow2 exact)
  - final F0 complex multiply + output DMA
Sharding: 8 cores = 4 wavelengths x 2 column-halves. Zero communication.
"""
import math
import numpy as np

f32 = np.float32
f16 = np.float16
f64 = np.float64

# ---- static geometry (mirrors the problem spec) ----
H = 1024
O_H = 1024
N_WL = 4
DX = 100e-6
ODX = 10e-6
ODY = 10e-6
Z = 0.05
TWO_PI = 2.0 * np.pi
M = 1024
P = 128
NB = 8          # partition blocks per plane
NQ = 4          # quadrant blocks
HN = 512        # half width

X_IN = np.linspace(-H * DX / 2, H * DX / 2, H).astype(f64)
X_OUT = np.linspace(-O_H * ODX / 2, O_H * ODX / 2, O_H).astype(f64)
SIGMA = np.concatenate([np.arange(512), np.arange(1023, 511, -1)])  # sigma(s)
C0 = Z / TWO_PI
J512 = np.arange(HN, dtype=f64)


def _pow2_below(x):
    return 2.0 ** math.floor(math.log2(x))


def _quad_planes(xg):
    r2 = xg[:512, None] ** 2 + xg[None, :512] ** 2 + Z * Z
    r = np.sqrt(r2)
    return r, 1.0 / r2, 1.0 / (r2 * r)


def host_prepare(field_real, field_imag, wavelengths):
    """Build per-core input maps + output assembly metadata. All f64 host math."""
    wls = np.asarray(wavelengths, f64)
    maxfield = float(max(np.abs(field_real).max(), np.abs(field_imag).max(), 1e-30))

    rq, i2q, i3q = _quad_planes(X_IN)
    ro, i2o, i3o = _quad_planes(X_OUT)

    perm_fields = {}
    for w in range(N_WL):
        fc = (np.asarray(field_real[0, w], f64)
              + 1j * np.asarray(field_imag[0, w], f64))
        perm_fields[w] = np.ascontiguousarray(fc[SIGMA][:, SIGMA])

    sg = SIGMA.astype(f64)
    in_maps = []
    meta = []
    ecache = {}
    for core in range(8):
        w, bh = core // 2, core % 2
        wl = f64(wls[w])
        Dm = wl * Z / DX
        fx1 = X_OUT[0] + Dm / 2
        fx2 = X_OUT[-1] + Dm / 2
        D1 = fx1 + (M * Dm + fx2 - fx1) / (2 * M)
        D2 = fx2 + (M * Dm + fx2 - fx1) / (2 * M)
        alpha = TWO_PI * (D2 - D1) / (M * Dm)
        beta = alpha - TWO_PI * D1 / Dm
        kwav = TWO_PI / wl
        gam1 = TWO_PI * (M - 1) * (D2 - D1) / (2 * Dm * M) - alpha
        gam0 = TWO_PI * (M - 1) * D1 / (2 * Dm) - alpha / 2

        s_w = Z * ODX * ODY * wl

        # --- input-plane RS kernel quad; U0 = field.F (f64) ---
        if (w, 'F') in ecache:
            Fq = ecache[(w, 'F')]
        else:
            phq = kwav * rq
            cq, sq = np.cos(phq), np.sin(phq)
            aa = i3q * C0
            bt = i2q * (kwav * C0)
            Fq = (aa * cq + bt * sq) + 1j * (aa * sq - bt * cq)
            ecache[(w, 'F')] = Fq
        A = perm_fields[w] * np.tile(Fq, (2, 2))

        # pow2 scales from exact column statistics so every fp16 stage sits in
        # the normal range with ~8x headroom to overflow (Cauchy-Schwarz bounds
        # the true step-1 max at 5.7x the 8-sigma estimate -> cast-safe).
        a2 = A.real ** 2 + A.imag ** 2
        s1_raw = 8.0 * math.sqrt(0.5 * float(a2.sum(axis=0).max()))
        fro = math.sqrt(float(a2.sum()))
        C_U = _pow2_below(8192.0 / s1_raw)
        C_B = _pow2_below(8192.0 / (8.0 * 0.7071 * C_U * fro))
        s_eff = s_w / (C_U * C_B)

        U0 = A * (C_U * C_B)
        u0r = f16(U0.real)
        u0i = f16(U0.imag)

        # --- chirp matrix E (per (w, bh)); halves differ by parity ---
        ekey = (w, bh)
        if ekey not in ecache:
            ph = np.empty((1024, 1024), f64)
            for h in range(2):
                par = (h + bh) % 2
                sgn = 1.0 - 2.0 * par
                base = 1023.0 * par
                sl = sgn * (alpha * sg + gam1)
                of = (alpha * sg + gam1) * base + beta * sg + gam0
                ph[:, HN * h:HN * (h + 1)] = sl[:, None] * J512[None, :] + of[:, None]
            er = f16(np.cos(ph))
            ei = f16(np.sin(ph))
            if bh == 0:
                er[0, 512] = 0.0
                ei[0, 512] = 0.0
            else:
                er[0, 0] = 0.0
                ei[0, 0] = 0.0
            ecache[ekey] = (er, ei)
        er, ei = ecache[ekey]

        # --- output-plane kernel quad, final scale folded; rescale to fp16
        # range with a pow2 undone on the host in assemble() ---
        pho = kwav * ro
        co, so = np.cos(pho), np.sin(pho)
        aao = i3o * (C0 * s_eff)
        bto = i2o * (kwav * C0 * s_eff)
        f0r_raw = aao * co + bto * so
        f0i_raw = aao * so - bto * co
        f0max = max(np.abs(f0r_raw).max(), np.abs(f0i_raw).max(), 1e-300)
        C_F0 = _pow2_below(1.0 / f0max)
        f0r = f16(f0r_raw * C_F0)
        f0i = f16(f0i_raw * C_F0)

        in_maps.append({
            "u0": np.ascontiguousarray(np.concatenate([u0r, u0i], axis=1)),
            "el": np.ascontiguousarray(np.concatenate([er[:, :HN], ei[:, :HN]], axis=1)),
            "er2": np.ascontiguousarray(np.concatenate([er[:, HN:], ei[:, HN:]], axis=1)),
            "f0": np.ascontiguousarray(np.concatenate([f0r, f0i], axis=1)),
        })
        rmap = SIGMA[(np.arange(1024) + 512 * bh) % 1024]
        cmap = np.arange(512) if bh == 0 else 1023 - np.arange(512)
        meta.append((w, rmap, cmap, 1.0 / C_F0))
    return in_maps, meta


def assemble(results, meta):
    out = np.zeros((1, N_WL, O_H, O_H), np.complex64)
    for core, (w, rmap, cmap, inv_cf0) in enumerate(meta):
        y = (results[core]["yre"].astype(f32)
             + 1j * results[core]["yim"].astype(f32)) * f32(inv_cf0)
        out[0, w][np.ix_(cmap, rmap)] = y.T
    return out


# ---------------- golden (numpy) model of the device program ----------------

def golden_core(inp):
    er = np.concatenate([inp["el"][:, :HN], inp["er2"][:, :HN]], axis=1)
    ei = np.concatenate([inp["el"][:, HN:], inp["er2"][:, HN:]], axis=1)
    es = f16(er.astype(f32) + ei.astype(f32))
    u0r, u0i = inp["u0"][:, :1024], inp["u0"][:, 1024:]
    u0s = f16(u0r.astype(f32) + u0i.astype(f32))

    def mm(A, B):
        return A.astype(f32).T @ B.astype(f32)

    # step 1 (Karatsuba): P1 = U0r^T ErL, P2 = U0i^T EiL, P3 = U0s^T EsL
    P1 = mm(u0r, er[:, :HN])
    P2 = mm(u0i, ei[:, :HN])
    P3 = mm(u0s, es[:, :HN])
    X1r = f16(P1 - P2)
    t01 = f32(P1 + P2)
    X1i = f16(P3 - t01)
    X1s = f16(X1r.astype(f32) + X1i.astype(f32))

    Q1 = mm(er, X1r)
    Q2 = mm(ei, X1i)
    Q3 = mm(es, X1s)
    X2r = f16(Q1 - Q2)
    t01b = f32(Q1 + Q2)
    X2i = f16(Q3 - t01b)

    F0r = np.concatenate([inp["f0"][:, :HN]] * 2, axis=0)
    F0i = np.concatenate([inp["f0"][:, HN:]] * 2, axis=0)
    t1 = f16(F0r.astype(f32) * X2r.astype(f32))
    t2 = f16(F0i.astype(f32) * X2i.astype(f32))
    Yre = f16(t1.astype(f32) - t2.astype(f32))
    t3 = f16(F0r.astype(f32) * X2i.astype(f32))
    t4 = f16(F0i.astype(f32) * X2r.astype(f32))
    Yim = f16(t3.astype(f32) + t4.astype(f32))
    return {"yre": Yre, "yim": Yim}


def golden(field_real, field_imag, wavelengths):
    in_maps, meta = host_prepare(field_real, field_imag, wavelengths)
    results = [golden_core(m) for m in in_maps]
    return assemble(results, meta)


# ---------------- bass program ----------------

_PROGRAM = None


def build_program():
    import concourse.bass as bass
    import concourse.tile as tile
    import concourse.mybir as mybir
    from concourse import bacc

    dt = mybir.dt
    ALU = mybir.AluOpType

    nc = bacc.Bacc("TRN2", target_bir_lowering=False, debug=False, num_devices=8)

    u0_d = nc.dram_tensor("u0", [1024, 2048], dt.float16, kind="ExternalInput").ap()
    el_d = nc.dram_tensor("el", [1024, 1024], dt.float16, kind="ExternalInput").ap()
    er2_d = nc.dram_tensor("er2", [1024, 1024], dt.float16, kind="ExternalInput").ap()
    f0_d = nc.dram_tensor("f0", [NQ * P, 1024], dt.float16, kind="ExternalInput").ap()
    yre = nc.dram_tensor("yre", [1024, HN], dt.float16, kind="ExternalOutput").ap()
    yim = nc.dram_tensor("yim", [1024, HN], dt.float16, kind="ExternalOutput").ap()

    with tile.TileContext(nc) as tc:
      with tc.tile_pool(name="persist", bufs=1) as pp, \
           tc.tile_pool(name="psum", bufs=1, space="PSUM") as pspool, \
           tc.tile_pool(name="tmp", bufs=4) as tp:

        # merged r|i tiles (one wide DMA per block -> 2-4KB lines, few descriptors)
        U0RI = [pp.tile([P, 2048], dt.float16, tag=f"U0RI{t}", name=f"U0RI{t}") for t in range(NB)]
        ELRI = [pp.tile([P, 1024], dt.float16, tag=f"ELRI{t}", name=f"ELRI{t}") for t in range(NB)]
        ERRI = [pp.tile([P, 1024], dt.float16, tag=f"ERRI{t}", name=f"ERRI{t}") for t in range(NB)]
        F0RI = [pp.tile([P, 1024], dt.float16, tag=f"F0RI{q}", name=f"F0RI{q}") for q in range(NQ)]
        U0r = [U0RI[t][:, 0:1024] for t in range(NB)]
        U0i = [U0RI[t][:, 1024:2048] for t in range(NB)]
        ErL = [ELRI[t][:, 0:HN] for t in range(NB)]
        EiL = [ELRI[t][:, HN:1024] for t in range(NB)]
        ErR = [ERRI[t][:, 0:HN] for t in range(NB)]
        EiR = [ERRI[t][:, HN:1024] for t in range(NB)]
        F0r = [F0RI[q][:, 0:HN] for q in range(NQ)]
        F0i = [F0RI[q][:, HN:1024] for q in range(NQ)]
        U0s = [pp.tile([P, 1024], dt.float16, tag=f"U0s{t}", name=f"U0s{t}") for t in range(NB)]
        EsL = [pp.tile([P, HN], dt.float16, tag=f"EsL{t}", name=f"EsL{t}") for t in range(NB)]
        EsR = [pp.tile([P, HN], dt.float16, tag=f"EsR{t}", name=f"EsR{t}") for t in range(NB)]
        X1r = [pp.tile([P, HN], dt.float16, tag=f"X1r{t}", name=f"X1r{t}") for t in range(NB)]
        X1i = [pp.tile([P, HN], dt.float16, tag=f"X1i{t}", name=f"X1i{t}") for t in range(NB)]
        X1s = [pp.tile([P, HN], dt.float16, tag=f"X1s{t}", name=f"X1s{t}") for t in range(NB)]

        # PE warmup junk (gets HAM to 8/8 while input DMA streams)
        wlhs = pp.tile([P, P], dt.float16, tag="wlhs", name="wlhs")
        wrhs = pp.tile([P, HN], dt.float16, tag="wrhs", name="wrhs")
        nc.gpsimd.memset(wlhs[:], 0.0)
        nc.gpsimd.memset(wrhs[:], 0.0)

        # ---- input DMA issue (critical 6MB prefix only; E-right/F0 are
        # gated behind step-1 progress on the scalar queue so they don't
        # steal HBM bandwidth from the prefix) ----
        # byte-balanced across the two issue queues, interleaved per kt so
        # each kt group (U0 block + E-left block) completes together
        for t in range(NB):
            sl = slice(P * t, P * (t + 1))
            if t % 2 == 0:
                nc.sync.dma_start(U0RI[t][:], u0_d[sl, :])
                nc.gpsimd.dma_start(ELRI[t][:], el_d[sl, :])
            else:
                nc.gpsimd.dma_start(U0RI[t][:], u0_d[sl, :])
                nc.sync.dma_start(ELRI[t][:], el_d[sl, :])

        # warmup matmuls on banks 6,7 (real accumulations there start later)
        _wn = [0]

        def warmup(n):
            for _ in range(n):
                i = _wn[0]
                _wn[0] += 1
                wp = pspool.tile([P, HN], dt.float32, tag=f"ps{6 + i % 2}", name=f"wps{i}")
                nc.tensor.matmul(wp[:], lhsT=wlhs[:], rhs=wrhs[:], start=True, stop=True)

        warmup(16)
        FILL_TAG = [6]

        # DVE: derived sums as inputs land
        for t in range(NB):
            nc.vector.tensor_tensor(out=U0s[t][:], in0=U0r[t], in1=U0i[t], op=ALU.add)
            nc.vector.tensor_tensor(out=EsL[t][:], in0=ErL[t], in1=EiL[t], op=ALU.add)

        # ---- step 1: X1 = U0^T EhL, Karatsuba ----
        # pass0 (kt-outer, DMA-paced): mt0+mt1 fully (banks 0-5) plus mt2's
        # P1/P2 (banks 6,7) -> 8 real matmuls per kt group keep the PE busy
        # through the DMA window; remaining sweeps run part-outer afterwards.
        def s1_combine(mt, p1, p2, p3):
            # DVE may read only ONE psum operand per op; ScalarE stages p2.
            p2c = tp.tile([P, HN], dt.float32, tag="p2c", name=f"p2c_{mt}")
            t01 = tp.tile([P, HN], dt.float32, tag="t01", name=f"t01_{mt}")
            nc.scalar.mul(p2c[:], p2[:], 1.0)
            nc.vector.tensor_tensor(out=X1r[mt][:], in0=p1[:], in1=p2c[:], op=ALU.subtract)
            nc.vector.tensor_tensor(out=t01[:], in0=p1[:], in1=p2c[:], op=ALU.add)
            nc.vector.tensor_tensor(out=X1i[mt][:], in0=p3[:], in1=t01[:], op=ALU.subtract)
            nc.vector.tensor_tensor(out=X1s[mt][:], in0=X1r[mt][:], in1=X1i[mt][:], op=ALU.add)

        def s1_gated_dma(p):
            for t in (2 * p, 2 * p + 1):
                sl = slice(P * t, P * (t + 1))
                nc.scalar.dma_start(ERRI[t][:], er2_d[sl, :])
            if p >= 2:
                for q in (2 * (p - 2), 2 * (p - 2) + 1):
                    sl = slice(P * q, P * (q + 1))
                    nc.scalar.dma_start(F0RI[q][:], f0_d[sl, :])

        def s1psum(bank, nm):
            return pspool.tile([P, HN], dt.float32, tag=f"ps{bank}", name=nm)

        SRC = (U0r, ErL), (U0i, EiL), (U0s, EsL)
        ps = {}
        for mt, banks in ((0, (0, 1, 2)), (1, (3, 4, 5)), (2, (6, 7, 0)),
                          (3, (1, 2, 3)), (4, (4, 5, 6)), (5, (7, 0, 1)),
                          (6, (2, 3, 4)), (7, (5, 6, 7))):
            for part in range(3):
                ps[(mt, part)] = s1psum(banks[part], f"s1_{mt}_{part}")

        def sweep(mt, part):
            u0, el = SRC[part]
            msl = slice(P * mt, P * (mt + 1))
            for kt in range(NB):
                nc.tensor.matmul(ps[(mt, part)][:], lhsT=u0[kt][:, msl], rhs=el[kt],
                                 start=(kt == 0), stop=(kt == NB - 1))

        for kt in range(NB):
            st, sp = (kt == 0), (kt == NB - 1)
            for mt, parts in ((0, (0, 1, 2)), (1, (0, 1, 2)), (2, (0, 1))):
                msl = slice(P * mt, P * (mt + 1))
                for part in parts:
                    u0, el = SRC[part]
                    nc.tensor.matmul(ps[(mt, part)][:], lhsT=u0[kt][:, msl], rhs=el[kt],
                                     start=st, stop=sp)
        s1_combine(0, ps[(0, 0)], ps[(0, 1)], ps[(0, 2)])
        s1_combine(1, ps[(1, 0)], ps[(1, 1)], ps[(1, 2)])
        s1_gated_dma(0)
        # phase B: ordered so each bank's first reuse trails its combine free
        # (b1 freed by q2c(mt0); b0 by mt0's DVE combine)
        sweep(3, 0)
        sweep(2, 2)
        s1_combine(2, ps[(2, 0)], ps[(2, 1)], ps[(2, 2)])
        sweep(3, 1)
        sweep(3, 2)
        s1_combine(3, ps[(3, 0)], ps[(3, 1)], ps[(3, 2)])
        s1_gated_dma(1)
        for mt in (4, 5, 6, 7):
            for part in range(3):
                sweep(mt, part)
            s1_combine(mt, ps[(mt, 0)], ps[(mt, 1)], ps[(mt, 2)])
            if mt in (5, 7):
                s1_gated_dma(mt // 2)

        # E-right sums on gpsimd (idle by now); needed from step-2 mt>=4
        for t in range(NB):
            nc.gpsimd.tensor_tensor(out=EsR[t][:], in0=ErR[t][:], in1=EiR[t][:], op=ALU.add)

        # ---- step 2: X2 = E^T X1; final y = F0 . X2 ----
        # part-outer (Q2 sweep, Q1 sweep, Q3 sweep) so the combine chain
        # starts before the group's last matmul; the last group's Q3 runs as
        # two half-bank sweeps so its combine overlaps the final matmuls.
        for mt in range(NB):
            b = (3 * mt) % 8
            q1 = pspool.tile([P, HN], dt.float32, tag=f"ps{b}", name=f"s2Q1_{mt}")
            q2 = pspool.tile([P, HN], dt.float32, tag=f"ps{(b+1) % 8}", name=f"s2Q2_{mt}")
            if mt < 4:
                Ers, Eis, Ess = ErL, EiL, EsL
                csl = slice(P * mt, P * (mt + 1))
            else:
                Ers, Eis, Ess = ErR, EiR, EsR
                csl = slice(P * (mt - 4), P * (mt - 3))
            q = mt % NQ
            x2r = tp.tile([P, HN], dt.float16, tag="x2r", name=f"x2r{mt}")
            q2c = tp.tile([P, HN], dt.float32, tag="p2c", name=f"q2c_{mt}")
            t01 = tp.tile([P, HN], dt.float32, tag="t01", name=f"t01b_{mt}")
            t1 = tp.tile([P, HN], dt.float16, tag="y1", name=f"y1_{mt}")
            t4 = tp.tile([P, HN], dt.float16, tag="y4", name=f"y4_{mt}")
            for kt in range(NB):
                nc.tensor.matmul(q2[:], lhsT=Eis[kt][:, csl], rhs=X1i[kt][:],
                                 start=(kt == 0), stop=(kt == NB - 1))
            nc.scalar.mul(q2c[:], q2[:], 1.0)
            for kt in range(NB):
                nc.tensor.matmul(q1[:], lhsT=Ers[kt][:, csl], rhs=X1r[kt][:],
                                 start=(kt == 0), stop=(kt == NB - 1))
            nc.vector.tensor_tensor(out=x2r[:], in0=q1[:], in1=q2c[:], op=ALU.subtract)
            nc.vector.tensor_tensor(out=t01[:], in0=q1[:], in1=q2c[:], op=ALU.add)
            if mt < NB - 1:
                nc.vector.tensor_tensor(out=t1[:], in0=x2r[:], in1=F0r[q], op=ALU.mult)
            else:
                nc.vector.tensor_tensor(out=t1[:, 0:HN // 2], in0=x2r[:, 0:HN // 2],
                                        in1=F0r[q][:, 0:HN // 2], op=ALU.mult)
                nc.vector.tensor_tensor(out=t1[:, HN // 2:], in0=x2r[:, HN // 2:],
                                        in1=F0r[q][:, HN // 2:], op=ALU.mult)
            nc.gpsimd.tensor_tensor(out=t4[:], in0=x2r[:], in1=F0i[q], op=ALU.mult)
            halves = ((0, HN),) if mt < NB - 1 else ((0, HN // 2), (HN // 2, HN))
            for hi, (c0, c1) in enumerate(halves):
                w = c1 - c0
                if mt < NB - 1:
                    q3 = pspool.tile([P, HN], dt.float32, tag=f"ps{(b+2) % 8}",
                                     name=f"s2Q3_{mt}")
                else:
                    q3 = pspool.tile([P, w], dt.float32,
                                     tag=f"ps{(b+2) % 8}" if hi == 0 else "ps0",
                                     name=f"s2Q3_{mt}_{hi}")
                for kt in range(NB):
                    nc.tensor.matmul(q3[:], lhsT=Ess[kt][:, csl], rhs=X1s[kt][:, c0:c1],
                                     start=(kt == 0), stop=(kt == NB - 1))
                x2i = tp.tile([P, w], dt.float16, tag="x2i", name=f"x2i{mt}_{hi}")
                t2 = tp.tile([P, w], dt.float16, tag="y2", name=f"y2_{mt}_{hi}")
                t3 = tp.tile([P, w], dt.float16, tag="y3", name=f"y3_{mt}_{hi}")
                yr = tp.tile([P, w], dt.float16, tag="yr", name=f"yr{mt}_{hi}")
                yi = tp.tile([P, w], dt.float16, tag="yi", name=f"yi{mt}_{hi}")
                msl = slice(P * mt, P * (mt + 1))
                nc.vector.tensor_tensor(out=x2i[:], in0=q3[:], in1=t01[:, c0:c1], op=ALU.subtract)
                nc.vector.tensor_tensor(out=t2[:], in0=x2i[:], in1=F0i[q][:, c0:c1], op=ALU.mult)
                nc.vector.tensor_tensor(out=yr[:], in0=t1[:, c0:c1], in1=t2[:], op=ALU.subtract)
                if mt < NB - 1:
                    nc.sync.dma_start(yre[msl, :], yr[:])
                elif hi == 0:
                    nc.sync.dma_start(yre[msl, c0:c1], yr[:])
                else:
                    nc.scalar.dma_start(yre[msl, c0:c1], yr[:])
                nc.vector.tensor_tensor(out=t3[:], in0=x2i[:], in1=F0r[q][:, c0:c1], op=ALU.mult)
                nc.vector.tensor_tensor(out=yi[:], in0=t3[:], in1=t4[:, c0:c1], op=ALU.add)
                if mt < NB - 1:
                    nc.sync.dma_start(yim[msl, :], yi[:])
                elif hi == 0:
                    nc.gpsimd.dma_start(yim[msl, c0:c1], yi[:])
                else:
                    nc.sync.dma_start(yim[msl, c0:c1], yi[:])

    nc.compile()
    return nc


def get_program():
    global _PROGRAM
    if _PROGRAM is None:
        _PROGRAM = build_program()
    return _PROGRAM


def kernel(field_real, field_imag, wavelengths):
    field_real = np.asarray(field_real)
    field_imag = np.asarray(field_imag)
    wavelengths = np.asarray(wavelengths)
    in_maps, meta = host_prepare(field_real, field_imag, wavelengths)
    from concourse.bass_utils import run_bass_kernel_spmd
    nc = get_program()
    res = run_bass_kernel_spmd(nc, in_maps, core_ids=list(range(8)))
    return assemble(res.results, meta)


if __name__ == "__main__":
    import jax
    import reference as ref
    cpu = jax.devices("cpu")[0]
    with jax.default_device(cpu):
        inputs = {k: np.asarray(v) for k, v in ref.setup_inputs().items()}
        expected = np.asarray(ref.reference(**{k: jax.device_put(v, cpu) for k, v in inputs.items()}))
    got = golden(np.asarray(inputs["field_real"]), np.asarray(inputs["field_imag"]),
                 np.asarray(inputs["wavelengths"]))
    err = np.abs(got - expected)
    print(f"golden absmax err {err.max():.4g} rel {err.max() / np.abs(expected).max():.4g}")



# revision 8
# speedup vs baseline: 1.7163x; 1.7163x over previous
"""Trainium2 Bass kernel for nn_CZT_prop: chirp-z (Bluestein) optical propagation.

Math: per wavelength both CZT axes share the transform M = diag(q) Tc diag(p)
with Tc[c,k] = tau(c-k), tau(d) = exp(-i*aw*d^2/2) an EVEN Toeplitz kernel, so
Tc is centrosymmetric and block-diagonalizes: Tc = K blockdiag(T+, T-) K / 2,
T+/-[c,k] = tau(c-k) +/- tau(c+k-1023), size 512.  The 2D result
    out = F0' . (Tc V Tc^T),   V = (field.F) * outer(p,p)
splits into four half-size quadrant products G_ab = T_a Vt_ab T_b (a,b in {+,-})
where Vt_ab are the +/- fold combos of V (host-prepared).  This HALVES the
device MACs vs the dense formulation.  The recombine (16 flip-adds), the F0'
multiply, and a rank-1 correction for the reference's zero-padded corner entry
[1023,0] of Tc are all host-side assembly.

Device per core (= one (wavelength, a-class)): two chained Karatsuba complex
matmul stages, contraction 512:
    S_b[j,c] = sum_k Vt_ab[k,j] T_a[k,c]     (b in {self, other})
    G_ab[c,d] = sum_j S_b[j,c] T_b[j,d]
192 fp16 matmuls of [128x512]@K=128 = 98304 PE cycles.  Sharding: 8 cores =
4 wavelengths x 2 centro-classes.  Zero communication.
"""
import math
import numpy as np

f32 = np.float32
f16 = np.float16
f64 = np.float64

# ---- static geometry (mirrors the problem spec) ----
H = 1024
M = 1024
N_WL = 4
DX = 100e-6
ODX = 10e-6
ODY = 10e-6
Z = 0.05
TWO_PI = 2.0 * np.pi
P = 128
HN = 512
NT = 4          # 128-row tiles per 512
X_IN = np.linspace(-H * DX / 2, H * DX / 2, H).astype(f64)
X_OUT = np.linspace(-M * ODX / 2, M * ODX / 2, M).astype(f64)


def _pow2_below(x):
    return 2.0 ** math.floor(math.log2(x))


def czt_factors(wl):
    """q[c], p[k], aw for the 1-axis CZT: out[c] = q[c] sum_k tau(c-k) p[k] x[k]."""
    Dm = wl * Z / DX
    f1 = X_OUT[0] + Dm / 2
    f2 = X_OUT[-1] + Dm / 2
    D1 = f1 + (M * Dm + f2 - f1) / (2 * M)
    D2 = f2 + (M * Dm + f2 - f1) / (2 * M)
    alpha_A = TWO_PI * D1 / Dm
    aw = -TWO_PI * (D1 - D2) / (M * Dm)
    k = np.arange(H, dtype=f64)
    c = np.arange(M, dtype=f64)
    h = lambda x: np.exp(1j * aw * x * x / 2)
    l = c / M * (D2 - D1) + D1
    m_shift = np.exp(-1j * TWO_PI * l * (-H / 2 + 0.5) / Dm)
    q = m_shift * h(c) * np.exp(-1j * aw * c) * np.exp(-1j * aw / 2)
    p = np.exp(-1j * alpha_A * k) * h(k) * np.exp(1j * aw * k)
    return q, p, aw


def _rs_kernel_full(xg, wl):
    """RS transfer kernel on the full plane via its 4-fold quad symmetry."""
    kv = TWO_PI / wl
    xh = xg[:HN]
    r2 = xh[:, None] ** 2 + xh[None, :] ** 2 + Z * Z
    r = np.sqrt(r2)
    aa = (Z / TWO_PI) / (r2 * r)
    bt = (kv * Z / TWO_PI) / r2
    ph = kv * r
    cq, sq = np.cos(ph), np.sin(ph)
    Fq = (aa * cq + bt * sq) + 1j * (aa * sq - bt * cq)
    return np.block([[Fq, Fq[:, ::-1]], [Fq[::-1, :], Fq[::-1, ::-1]]])


def host_prepare(field_real, field_imag, wavelengths):
    """Per-core device inputs + assembly metadata.  All f64 host math."""
    wls = np.asarray(wavelengths, f64)
    Jr = np.arange(HN)[::-1]
    in_maps = [None] * 8
    meta = []
    for w in range(N_WL):
        wl = f64(wls[w])
        q, p, aw = czt_factors(wl)
        tau = lambda d: np.exp(-1j * aw * np.asarray(d, f64) ** 2 / 2)
        F = _rs_kernel_full(X_IN, wl)
        F0 = _rs_kernel_full(X_OUT, wl)
        U = (np.asarray(field_real[0, w], f64)
             + 1j * np.asarray(field_imag[0, w], f64)) * F
        V = U * np.outer(p, p)

        cc = np.arange(HN, dtype=f64)[:, None]
        kk = np.arange(HN, dtype=f64)[None, :]
        tnear = tau(cc - kk)
        tfar = tau(cc + kk - (H - 1))
        Tp = tnear + tfar
        Tm = tnear - tfar

        V11 = V[:HN, :HN]; V12 = V[:HN, HN:]
        V21 = V[HN:, :HN]; V22 = V[HN:, HN:]
        A12 = V12[:, Jr]; A21 = V21[Jr, :]; A22 = V22[Jr][:, Jr]
        Vt = {('+', '+'): (V11 + A12 + A21 + A22) / 4,
              ('+', '-'): (V11 - A12 + A21 - A22) / 4,
              ('-', '+'): (V11 + A12 - A21 - A22) / 4,
              ('-', '-'): (V11 - A12 - A21 + A22) / 4}

        # pow2 scaling: one scale per (a,b) block, chosen so BOTH stage
        # outputs stay in fp16 range with ~8x headroom (stage-2 dominates).
        mt2 = {'+': float(np.mean(np.abs(Tp) ** 2)),
               '-': float(np.mean(np.abs(Tm) ** 2))}
        scales = {}
        for ab, Vab in Vt.items():
            fro2 = float(np.sum(np.abs(Vab) ** 2))
            s2_raw = 8.0 * math.sqrt(0.5 * mt2[ab[0]] * mt2[ab[1]] * fro2)
            scales[ab] = _pow2_below(8192.0 / max(s2_raw, 1e-300))

        Tdev = {'+': np.ascontiguousarray(
                    np.concatenate([f16(Tp.real), f16(Tp.imag)], axis=1)),
                '-': np.ascontiguousarray(
                    np.concatenate([f16(Tm.real), f16(Tm.imag)], axis=1))}
        for ai, a in enumerate(('+', '-')):
            b_self, b_oth = a, ('-' if a == '+' else '+')
            vs = Vt[(a, b_self)] * scales[(a, b_self)]
            vo = Vt[(a, b_oth)] * scales[(a, b_oth)]
            in_maps[2 * w + ai] = {
                "ts": Tdev[a],
                "to": Tdev[b_oth],
                "vs": np.ascontiguousarray(
                    np.concatenate([f16(vs.real), f16(vs.imag)], axis=1)),
                "vo": np.ascontiguousarray(
                    np.concatenate([f16(vo.real), f16(vo.imag)], axis=1)),
            }

        # rank-1 corner correction (f64, exact): out = Tc_ref V Tc_ref^T,
        # Tc_ref = Tc - s e_1023 e_0^T.  Row/col vectors via T+/- blocks:
        # T11 = (Tp+Tm)/2, H = (Tp-Tm)/2 (T12 = H J, T21 = J H, T22 = J T11 J)
        s = tau(f64(H - 1))
        T11 = (Tp + Tm) / 2
        Hh = (Tp - Tm) / 2
        v1 = V[0, :HN]; v2r = V[0, HN:][Jr]
        row = np.empty(M, np.complex128)
        row[:HN] = v1 @ T11 + v2r @ Hh
        row[HN:] = (v1 @ Hh + v2r @ T11)[Jr]
        u1 = V[:HN, 0]; u2r = V[HN:, 0][Jr]
        col = np.empty(M, np.complex128)
        col[:HN] = T11 @ u1 + Hh @ u2r
        col[HN:] = (Hh @ u1 + T11 @ u2r)[Jr]
        # F0' with all diagonal factors + global scale folded
        F0p = F0 * np.outer(q, q) * (Z * ODX * ODY * wl)
        meta.append({
            "w": w,
            "scales": scales,
            "F0p": F0p.astype(np.complex128),
            "corr_row": s * row, "corr_col": s * col,
            "corr_s": s * s * V[0, 0],
        })
    return in_maps, meta


def assemble(results, meta):
    out = np.zeros((1, N_WL, M, M), np.complex64)
    Jr = np.arange(HN)[::-1]
    for md in meta:
        w = md["w"]
        qs = {}
        for ai, a in enumerate(('+', '-')):
            r = results[2 * w + ai]
            b_self, b_oth = a, ('-' if a == '+' else '+')
            for key, ab in (("gs", (a, b_self)), ("go", (a, b_oth))):
                g = r[key].astype(f32)
                qs[ab] = (g[:, :HN].astype(f64) + 1j * g[:, HN:].astype(f64)) \
                    / md["scales"][ab]
        Q1 = qs[('+', '+')]; Q2 = qs[('+', '-')]
        Q3 = qs[('-', '+')]; Q4 = qs[('-', '-')]
        Gf = np.empty((M, M), np.complex128)
        Gf[:HN, :HN] = Q1 + Q2 + Q3 + Q4
        Gf[:HN, HN:] = (Q1 - Q2 + Q3 - Q4)[:, Jr]
        Gf[HN:, :HN] = (Q1 + Q2 - Q3 - Q4)[Jr, :]
        Gf[HN:, HN:] = (Q1 - Q2 - Q3 + Q4)[Jr][:, Jr]
        Gf[M - 1, :] -= md["corr_row"]
        Gf[:, M - 1] -= md["corr_col"]
        Gf[M - 1, M - 1] += md["corr_s"]
        out[0, w] = (md["F0p"] * Gf).astype(np.complex64)
    return out


# ---------------- golden (numpy) model of the device program ----------------

def golden_core(inp):
    def split(x):
        return x[:, :HN], x[:, HN:]

    def karatsuba(Ar, Ai, As, Br, Bi, Bs):
        P1 = Ar.astype(f32).T @ Br.astype(f32)
        P2 = Ai.astype(f32).T @ Bi.astype(f32)
        P3 = As.astype(f32).T @ Bs.astype(f32)
        Xr = f16(P1 - P2)
        Xi = f16(P3 - f32(P1 + P2))
        return Xr, Xi

    tsr, tsi = split(inp["ts"]); tss = f16(tsr.astype(f32) + tsi.astype(f32))
    tor_, toi = split(inp["to"]); tos = f16(tor_.astype(f32) + toi.astype(f32))
    out = {}
    for key, vkey, (br, bi, bs) in (("gs", "vs", (tsr, tsi, tss)),
                                    ("go", "vo", (tor_, toi, tos))):
        vr, vi = split(inp[vkey]); vv = f16(vr.astype(f32) + vi.astype(f32))
        Sr, Si = karatsuba(vr, vi, vv, tsr, tsi, tss)
        Ss = f16(Sr.astype(f32) + Si.astype(f32))
        Gr, Gi = karatsuba(Sr, Si, Ss, br, bi, bs)
        out[key] = np.concatenate([Gr, Gi], axis=1)
    return out


def golden(field_real, field_imag, wavelengths):
    in_maps, meta = host_prepare(field_real, field_imag, wavelengths)
    results = [golden_core(m) for m in in_maps]
    return assemble(results, meta)


# ---------------- bass program ----------------

_PROGRAM = None


def build_program():
    import concourse.bass as bass
    import concourse.tile as tile
    import concourse.mybir as mybir
    from concourse import bacc

    dt = mybir.dt
    ALU = mybir.AluOpType

    nc = bacc.Bacc("TRN2", target_bir_lowering=False, debug=False, num_devices=8)

    ts_d = nc.dram_tensor("ts", [HN, 1024], dt.float16, kind="ExternalInput").ap()
    to_d = nc.dram_tensor("to", [HN, 1024], dt.float16, kind="ExternalInput").ap()
    vs_d = nc.dram_tensor("vs", [HN, 1024], dt.float16, kind="ExternalInput").ap()
    vo_d = nc.dram_tensor("vo", [HN, 1024], dt.float16, kind="ExternalInput").ap()
    gs_d = nc.dram_tensor("gs", [HN, 1024], dt.float16, kind="ExternalOutput").ap()
    go_d = nc.dram_tensor("go", [HN, 1024], dt.float16, kind="ExternalOutput").ap()

    with tile.TileContext(nc) as tc:
      with tc.tile_pool(name="persist", bufs=1) as pp, \
           tc.tile_pool(name="psum", bufs=1, space="PSUM") as pspool, \
           tc.tile_pool(name="tmp", bufs=4) as tp:

        TS = [pp.tile([P, 1024], dt.float16, tag=f"TS{t}", name=f"TS{t}") for t in range(NT)]
        TO = [pp.tile([P, 1024], dt.float16, tag=f"TO{t}", name=f"TO{t}") for t in range(NT)]
        VS = [pp.tile([P, 1024], dt.float16, tag=f"VS{t}", name=f"VS{t}") for t in range(NT)]
        VO = [pp.tile([P, 1024], dt.float16, tag=f"VO{t}", name=f"VO{t}") for t in range(NT)]
        TSr = [TS[t][:, 0:HN] for t in range(NT)]
        TSi = [TS[t][:, HN:1024] for t in range(NT)]
        TOr = [TO[t][:, 0:HN] for t in range(NT)]
        TOi = [TO[t][:, HN:1024] for t in range(NT)]
        VSr = [VS[t][:, 0:HN] for t in range(NT)]
        VSi = [VS[t][:, HN:1024] for t in range(NT)]
        VOr = [VO[t][:, 0:HN] for t in range(NT)]
        VOi = [VO[t][:, HN:1024] for t in range(NT)]
        TSs = [pp.tile([P, HN], dt.float16, tag=f"TSs{t}", name=f"TSs{t}") for t in range(NT)]
        TOs = [pp.tile([P, HN], dt.float16, tag=f"TOs{t}", name=f"TOs{t}") for t in range(NT)]
        VSs = [pp.tile([P, HN], dt.float16, tag=f"VSs{t}", name=f"VSs{t}") for t in range(NT)]
        VOs = [pp.tile([P, HN], dt.float16, tag=f"VOs{t}", name=f"VOs{t}") for t in range(NT)]
        # S tiles for both b-blocks: r, i, s
        Sr = {b: [pp.tile([P, HN], dt.float16, tag=f"S{b}r{t}", name=f"S{b}r{t}")
                  for t in range(NT)] for b in "so"}
        Si = {b: [pp.tile([P, HN], dt.float16, tag=f"S{b}i{t}", name=f"S{b}i{t}")
                  for t in range(NT)] for b in "so"}
        Ss = {b: [pp.tile([P, HN], dt.float16, tag=f"S{b}s{t}", name=f"S{b}s{t}")
                  for t in range(NT)] for b in "so"}

        # PE warmup junk (p-state ramp while input DMA streams)
        wlhs = pp.tile([P, P], dt.float16, tag="wlhs", name="wlhs")
        wrhs = pp.tile([P, HN], dt.float16, tag="wrhs", name="wrhs")
        nc.gpsimd.memset(wlhs[:], 0.0)
        nc.gpsimd.memset(wrhs[:], 0.0)

        # ---- input DMA issue: ts/vs interleaved per kt (stage-1 critical),
        # then vo, then to (needed only for stage-2-other) ----
        for t in range(NT):
            sl = slice(P * t, P * (t + 1))
            nc.sync.dma_start(TS[t][:], ts_d[sl, :])
            nc.gpsimd.dma_start(VS[t][:], vs_d[sl, :])
        for t in range(NT):
            sl = slice(P * t, P * (t + 1))
            nc.sync.dma_start(VO[t][:], vo_d[sl, :])
            nc.gpsimd.dma_start(TO[t][:], to_d[sl, :])

        _wn = [0]

        def warmup(n):
            for _ in range(n):
                i = _wn[0]
                _wn[0] += 1
                wp = pspool.tile([P, HN], dt.float32, tag=f"ps{6 + i % 2}", name=f"wps{i}")
                nc.tensor.matmul(wp[:], lhsT=wlhs[:], rhs=wrhs[:], start=True, stop=True)

        warmup(14)

        # derived sum planes as inputs land (vector: 4x fp16 mode)
        for t in range(NT):
            nc.vector.tensor_tensor(out=TSs[t][:], in0=TSr[t], in1=TSi[t], op=ALU.add)
            nc.vector.tensor_tensor(out=VSs[t][:], in0=VSr[t], in1=VSi[t], op=ALU.add)
        for t in range(NT):
            nc.gpsimd.tensor_tensor(out=VOs[t][:], in0=VOr[t], in1=VOi[t], op=ALU.add)
            nc.gpsimd.tensor_tensor(out=TOs[t][:], in0=TOr[t], in1=TOi[t], op=ALU.add)

        # ---- stages ----
        # group = (phase, mt): phase 0 = S_self, 1 = S_other, 2 = G_self, 3 = G_other
        # per group 3 psum banks rotating.
        gctr = [0]

        def run_group(phase, mt, out_r, out_i, out_s, lhs_parts, rhs_parts,
                      split_tail=False):
            """lhs_parts/rhs_parts: (r, i, s) lists of NT tiles (lhsT source &
            rhs).  Emits P2, P1, P3 sweeps + combine."""
            g = gctr[0]
            gctr[0] += 1
            b0 = (3 * g) % 8
            lr, li, ls = lhs_parts
            rr, ri, rs = rhs_parts
            msl = slice(P * mt, P * (mt + 1))
            p2 = pspool.tile([P, HN], dt.float32, tag=f"ps{b0}", name=f"p2_{phase}_{mt}")
            p1 = pspool.tile([P, HN], dt.float32, tag=f"ps{(b0 + 1) % 8}", name=f"p1_{phase}_{mt}")
            for kt in range(NT):
                nc.tensor.matmul(p2[:], lhsT=li[kt][:, msl], rhs=ri[kt],
                                 start=(kt == 0), stop=(kt == NT - 1))
            p2c = tp.tile([P, HN], dt.float32, tag="p2c", name=f"p2c_{phase}_{mt}")
            nc.scalar.mul(p2c[:], p2[:], 1.0)
            for kt in range(NT):
                nc.tensor.matmul(p1[:], lhsT=lr[kt][:, msl], rhs=rr[kt],
                                 start=(kt == 0), stop=(kt == NT - 1))
            t01 = tp.tile([P, HN], dt.float32, tag="t01", name=f"t01_{phase}_{mt}")
            nc.vector.tensor_tensor(out=out_r, in0=p1[:], in1=p2c[:], op=ALU.subtract)
            nc.vector.tensor_tensor(out=t01[:], in0=p1[:], in1=p2c[:], op=ALU.add)
            halves = ((0, HN),) if not split_tail else ((0, HN // 2), (HN // 2, HN))
            for hi, (c0, c1) in enumerate(halves):
                wdt = c1 - c0
                full = (wdt == HN)
                p3 = pspool.tile([P, wdt], dt.float32,
                                 tag=f"ps{(b0 + 2) % 8}" if hi == 0 else f"ps{(b0 + 3) % 8}",
                                 name=f"p3_{phase}_{mt}_{hi}")
                for kt in range(NT):
                    nc.tensor.matmul(p3[:], lhsT=ls[kt][:, msl],
                                     rhs=rs[kt] if full else rs[kt][:, c0:c1],
                                     start=(kt == 0), stop=(kt == NT - 1))
                nc.vector.tensor_tensor(out=out_i if full else out_i[:, c0:c1],
                                        in0=p3[:],
                                        in1=t01[:] if full else t01[:, c0:c1],
                                        op=ALU.subtract)
                if out_s is not None:
                    nc.vector.tensor_tensor(out=out_s if full else out_s[:, c0:c1],
                                            in0=out_r if full else out_r[:, c0:c1],
                                            in1=out_i if full else out_i[:, c0:c1],
                                            op=ALU.add)

        TSsA = [TSs[t][:] for t in range(NT)]
        TOsA = [TOs[t][:] for t in range(NT)]
        VSsA = [VSs[t][:] for t in range(NT)]
        VOsA = [VOs[t][:] for t in range(NT)]
        SrA = {b: [Sr[b][t][:] for t in range(NT)] for b in "so"}
        SiA = {b: [Si[b][t][:] for t in range(NT)] for b in "so"}
        SsA = {b: [Ss[b][t][:] for t in range(NT)] for b in "so"}

        # stage 1 self: S_s[j,c] = sum_k VS[k,j] TS[k,c]
        for mt in range(NT):
            run_group(0, mt, SrA['s'][mt], SiA['s'][mt], SsA['s'][mt],
                      (VSr, VSi, VSsA), (TSr, TSi, TSsA))
        # stage 1 other
        for mt in range(NT):
            run_group(1, mt, SrA['o'][mt], SiA['o'][mt], SsA['o'][mt],
                      (VOr, VOi, VOsA), (TSr, TSi, TSsA))

        # stage 2: G[c,d] = sum_j S[j,c] T[j,d]; pack r|i into one out tile
        def stage2(phase, b, rhs_parts, out_d, dma_engines):
            for mt in range(NT):
                gtile = tp.tile([P, 1024], dt.float16, tag="gout", name=f"g_{phase}_{mt}")
                run_group(phase, mt, gtile[:, 0:HN], gtile[:, HN:1024], None,
                          (SrA[b], SiA[b], SsA[b]), rhs_parts,
                          split_tail=(phase == 3 and mt == NT - 1))
                msl = slice(P * mt, P * (mt + 1))
                eng = dma_engines[mt % len(dma_engines)]
                eng(out_d[msl, :], gtile[:])

        stage2(2, 's', (TSr, TSi, TSsA), gs_d,
               [nc.sync.dma_start, nc.scalar.dma_start])
        stage2(3, 'o', (TOr, TOi, TOsA), go_d,
               [nc.sync.dma_start, nc.scalar.dma_start])

    nc.compile()
    return nc


def get_program():
    global _PROGRAM
    if _PROGRAM is None:
        _PROGRAM = build_program()
    return _PROGRAM


def kernel(field_real, field_imag, wavelengths):
    field_real = np.asarray(field_real)
    field_imag = np.asarray(field_imag)
    wavelengths = np.asarray(wavelengths)
    in_maps, meta = host_prepare(field_real, field_imag, wavelengths)
    from concourse.bass_utils import run_bass_kernel_spmd
    nc = get_program()
    res = run_bass_kernel_spmd(nc, in_maps, core_ids=list(range(8)))
    return assemble(res.results, meta)


if __name__ == "__main__":
    import jax
    import reference as ref
    cpu = jax.devices("cpu")[0]
    with jax.default_device(cpu):
        inputs = {k: np.asarray(v) for k, v in ref.setup_inputs().items()}
        expected = np.asarray(ref.reference(**{k: jax.device_put(v, cpu)
                                               for k, v in inputs.items()}))
    got = golden(np.asarray(inputs["field_real"]), np.asarray(inputs["field_imag"]),
                 np.asarray(inputs["wavelengths"]))
    err = np.abs(got - expected)
    print(f"golden absmax err {err.max():.4g} rel {err.max() / np.abs(expected).max():.4g}")
